# revision 34
# baseline (speedup 1.0000x reference)
"""Trainium2 Bass kernel for the BinaryMechanismSSM problem.

Full inputs in, full outputs out. Internally: batch (128) sharded 8 ways
(16 rows/core). Per core, a single fused pass:
  Projections (phase 1) are interleaved into the recurrence's idle PE/DVE/
  ACT windows: per 512-token tile, f32r matmuls compute bx0/bx1 (bias add
  + fp16 cast on DVE) and the gate planes gco = gcoef*sigmoid(x) (Pool)
  and g1m = sigmoid(-x) (ACT), packed into per-group-layout SBUF tiles and
  shipped to DRAM staging with one DMA each.
  Recurrence: T sequential steps, feature-block (j) pipelined. State lives
  as fp16 slices of a per-group staging tile stg_sb[p, t*64+j*16+b]
  (s[b, 128j+p] after step t). Per step: 4 psum tiles [128, 2*16] (one per
  feature block j); 1 fp16 identity matmul injects bx, 8 fp16 A-matmuls
  accumulate (each block consumes state block k in cyclic order ending
  with k=j); per-block tail tanh (ACT) -> mix/blend. Late-closing blocks
  j2/j3 run their blend on DVE, early blocks j0/j1 on GPSIMD; the (1-g)*s
  term is computed per half on the engine that produced that state half.
  One DMA per group ships 16 steps of states to DRAM; host re-layouts to
  [B, T+1, S].
"""
import numpy as np

B_FULL = 128
T_FULL = 1024
I_DIM = 256
S_DIM = 512
N_CORES = 8
B_LOC = B_FULL // N_CORES  # 16

_cache = {}


def _build(alpha: float, z: int, T: int):
    import concourse.bass as bass
    from concourse import bacc
    import concourse.mybir as mybir
    from concourse.tile import TileContext

    dt = mybir.dt
    AF = mybir.ActivationFunctionType
    ALU = mybir.AluOpType

    TOK = T * B_LOC          # tokens per core
    NTT = TOK // 512         # phase-1 token tiles (32 steps each)
    NG = T // 16             # phase-2 step groups
    NREC = 2 if z != 0 else 1
    NMAT = NREC + 1
    W = NREC * 16            # psum tile width per j block
    LAG = 2                  # tiles of projection lead

    # gate fold: st = gco * u + g1m * s, gco = gcoef * sigmoid, with
    #   alpha <= 0.5: gcoef = 1-alpha, u = ft0 + gam*ft1, gam = a/(1-a)
    #   alpha >  0.5: gcoef = alpha,   u = bet*ft0 + ft1, bet = (1-a)/a
    if NREC == 1:
        gcoef, mixc, mix_lo = 1.0, None, None
    elif alpha <= 0.5:
        gcoef, mixc, mix_lo = 1.0 - alpha, alpha / (1.0 - alpha), True
    else:
        gcoef, mixc, mix_lo = alpha, (1.0 - alpha) / alpha, False

    nc = bacc.Bacc("TRN2", target_bir_lowering=False, debug=False,
                   num_devices=N_CORES)

    xT_d = nc.declare_dram_parameter("xT", [2, 128, TOK], dt.float32r, isOutput=False)
    pw_d = nc.declare_dram_parameter("pw", [128, NMAT * 2 * 4 * 128], dt.float32r, isOutput=False)
    bias_d = nc.declare_dram_parameter("bias", [128, 4 * NMAT + 4], dt.float32, isOutput=False)
    aw_d = nc.declare_dram_parameter("aw", [128, NREC * 16 * 128], dt.float16, isOutput=False)
    s0_d = nc.declare_dram_parameter("s0T", [128, 64], dt.float16, isOutput=False)
    iden_d = nc.declare_dram_parameter("iden", [128, 128], dt.float16, isOutput=False)
    stg_d = nc.declare_dram_parameter("stg", [128, T * 64], dt.float16, isOutput=True)

    with TileContext(nc) as tc:
      with tc.tile_pool(name="dram", bufs=1, space="DRAM") as dpool:
        # bx staging: col = t*(NREC*64) + m*64 + j*16 + b
        pjb_d = dpool.tile([128, T * NREC * 64], dt.float16, tag="pjbd",
                           name="pjbd")
        # gate staging: col = t*128 + kind*64 + j*16 + b  (0=gco, 1=g1m)
        gat_d = dpool.tile([128, T * 128], dt.float16, tag="gatd", name="gatd")

        with (
            tc.tile_pool(name="wp", bufs=1) as wp,
            tc.tile_pool(name="p1x", bufs=3) as p1x,
            tc.tile_pool(name="p1o", bufs=3) as p1o,
            tc.tile_pool(name="p1ps", bufs=3, space="PSUM") as p1ps,
            tc.tile_pool(name="p2in", bufs=2) as p2in,
            tc.tile_pool(name="p2stg", bufs=2) as p2stg,
            tc.tile_pool(name="p2c", bufs=16) as p2c,
            tc.tile_pool(name="p2ps", bufs=2, space="PSUM") as p2ps,
        ):
            pw = wp.tile([128, NMAT * 2 * 4 * 128], dt.float32r)
            nc.sync.dma_start(pw[:], pw_d[:])
            bias = wp.tile([128, 4 * NMAT + 4], dt.float32)
            nc.sync.dma_start(bias[:], bias_d[:])
            gcoef_t = wp.tile([128, 256], dt.float16)
            nc.gpsimd.memset(gcoef_t[:], gcoef)
            aw = wp.tile([128, NREC * 16 * 128], dt.float16)
            nc.sync.dma_start(aw[:], aw_d[:])
            iden = wp.tile([128, 128], dt.float16)
            nc.sync.dma_start(iden[:], iden_d[:])
            s0sb = wp.tile([128, 64], dt.float16)
            nc.sync.dma_start(s0sb[:], s0_d[:])

            # ---------------- projection emitters ----------------
            p1st = {}

            def p1_prefetch(tt):
                if tt >= NTT:
                    return
                xt = p1x.tile([128, 2 * 512], dt.float32r, tag="xt")
                for i in range(2):
                    nc.sync.dma_start(xt[:, i * 512:(i + 1) * 512],
                                      xT_d[i, :, tt * 512:(tt + 1) * 512])
                p1st[tt] = {"xt": xt}

            def p1_unit(tt, u):
                """u in 0..NMAT*4-1 -> (mat, j): 2 matmuls + post ops."""
                if tt >= NTT:
                    return
                st = p1st[tt]
                if u == 0:
                    st["pjbpk"] = p1o.tile([128, 32 * NREC * 64], dt.float16,
                                           tag="pjbpk", name="pjbpk")
                    st["gatpk"] = p1o.tile([128, 32 * 128], dt.float16,
                                           tag="gatpk", name="gatpk")
                mat, j = divmod(u, 4)
                pjbpk_r = st["pjbpk"][:].rearrange(
                    "p (t m j b) -> p t m j b", t=32, m=NREC, j=4)
                gatpk_r = st["gatpk"][:].rearrange(
                    "p (t k j b) -> p t k j b", t=32, k=2, j=4)
                ps = p1ps.tile([128, 512], dt.float32, tag="pps")
                for i in range(2):
                    blk = ((mat * 2 + i) * 4 + j) * 128
                    nc.tensor.matmul(ps[:], pw[:, blk:blk + 128],
                                     st["xt"][:, i * 512:(i + 1) * 512],
                                     start=(i == 0), stop=(i == 1))
                psr = ps[:].rearrange("p (t b) -> p t b", t=32)
                bj = bias[:, mat * 4 + j:mat * 4 + j + 1]
                if mat == NMAT - 1:
                    g16 = p1o.tile([128, 512], dt.float16, tag="g16")
                    g16r = g16[:].rearrange("p (t b) -> p t b", t=32)
                    nbj = bias[:, 4 * NMAT + j:4 * NMAT + j + 1]
                    for h in range(2):
                        hs = slice(h * 16, (h + 1) * 16)
                        nc.scalar.activation(g16r[:, hs], psr[:, hs],
                                             AF.Sigmoid, bias=bj, scale=1.0)
                        nc.scalar.activation(gatpk_r[:, hs, 1, j, :],
                                             psr[:, hs], AF.Sigmoid,
                                             bias=nbj, scale=-1.0)
                        nc.gpsimd.tensor_tensor(
                            gatpk_r[:, hs, 0, j, :], g16r[:, hs],
                            gcoef_t[:].rearrange("p (t b) -> p t b", t=16),
                            ALU.mult)
                else:
                    for h in range(2):
                        hs = slice(h * 16, (h + 1) * 16)
                        nc.vector.tensor_scalar(pjbpk_r[:, hs, mat, j, :],
                                                psr[:, hs], bj, None, ALU.add)

            def p1_flush(tt):
                if tt >= NTT:
                    return
                st = p1st.pop(tt)
                GW32 = 32 * NREC * 64
                nc.sync.dma_start(pjb_d[:, tt * GW32:(tt + 1) * GW32],
                                  st["pjbpk"][:])
                nc.sync.dma_start(gat_d[:, tt * 4096:(tt + 1) * 4096],
                                  st["gatpk"][:])

            # prologue: first LAG tiles of projections
            NU = NMAT * 4
            for tt in range(min(LAG, NTT)):
                p1_prefetch(tt)
                for u in range(NU):
                    p1_unit(tt, u)
                p1_flush(tt)
            p1_prefetch(LAG)

            # ---------------- recurrence ----------------
            GW = 16 * NREC * 64
            prev_A = prev_B = None
            for g in range(NG):
                pjb = p2in.tile([128, GW], dt.float16, tag="pjb")
                nc.sync.dma_start(pjb[:], pjb_d[:, g * GW:(g + 1) * GW])
                pjb_r = pjb[:].rearrange("p (t m j b) -> p m j t b",
                                         t=16, m=NREC, j=4)
                gat = p2in.tile([128, 2048], dt.float16, tag="gat")
                nc.sync.dma_start(gat[:], gat_d[:, g * 2048:(g + 1) * 2048])

                # state halves in separate tiles so a read of one half never
                # waits on the other half's write
                stgA = p2stg.tile([128, 16 * 32], dt.float16, tag="stgA")
                stgB = p2stg.tile([128, 16 * 32], dt.float16, tag="stgB")

                def emit_m2(srcA, offA, srcB, offB, t):
                    m2a = p2c.tile([128, 32], dt.float16, tag="m2a",
                                   name="m2a")
                    nc.gpsimd.tensor_tensor(
                        m2a[:], srcA[:, offA:offA + 32],
                        gat[:, t * 128 + 64:t * 128 + 96], ALU.mult)
                    m2b = p2c.tile([128, 32], dt.float16, tag="m2b",
                                   name="m2b")
                    nc.vector.tensor_tensor(
                        m2b[:], srcB[:, offB:offB + 32],
                        gat[:, t * 128 + 96:t * 128 + 128], ALU.mult)
                    return m2a, m2b

                # projection work interleaved into this group
                tile_idx = g // 2 + LAG
                ubase = 0 if g % 2 == 0 else NU - NU // 2
                units = list(range(ubase, min(ubase + NU - NU // 2, NU)))

                if g == 0:
                    srcA, offA, srcB, offB = s0sb, 0, s0sb, 32
                else:
                    srcA, offA, srcB, offB = prev_A, 15 * 32, prev_B, 15 * 32
                m2a, m2b = emit_m2(srcA, offA, srcB, offB, 0)

                for tt in range(16):
                    if tt > 0:
                        srcA, offA = stgA, (tt - 1) * 32
                        srcB, offB = stgB, (tt - 1) * 32

                    def st_rhs(k):
                        if k < 2:
                            return srcA[:, offA + k * 16:offA + (k + 1) * 16]
                        return srcB[:, offB + (k - 2) * 16:
                                    offB + (k - 1) * 16]

                    # two psum pairs, cols (m, jj, b); idens + early matmuls
                    # (k=0,1) first, then the late batch (k=2,3)
                    psps = []
                    for p in range(2):
                        psp = p2ps.tile([128, 2 * W], dt.float32,
                                        tag=f"psp{p}", name=f"psp{p}")
                        psps.append(psp)
                        nc.tensor.matmul(
                            psp[:].rearrange("q (m j b) -> q m j b",
                                             m=NREC, j=2),
                            iden[:], pjb_r[:, :, 2 * p:2 * p + 2, tt, :],
                            start=True, stop=False)
                        for jj in range(2):
                            j = 2 * p + jj
                            for m in range(NREC):
                                for k in (0, 1):
                                    blk = (m * 16 + k * 4 + j) * 128
                                    nc.tensor.matmul(
                                        psp[:, (m * 2 + jj) * 16:
                                            (m * 2 + jj + 1) * 16],
                                        aw[:, blk:blk + 128], st_rhs(k),
                                        start=False, stop=False)
                    fts = []
                    for p in range(2):
                        psp = psps[p]
                        for jj in range(2):
                            j = 2 * p + jj
                            for m in range(NREC):
                                for k in (2, 3):
                                    blk = (m * 16 + k * 4 + j) * 128
                                    nc.tensor.matmul(
                                        psp[:, (m * 2 + jj) * 16:
                                            (m * 2 + jj + 1) * 16],
                                        aw[:, blk:blk + 128], st_rhs(k),
                                        start=False, stop=(k == 3))
                        ft = p2c.tile([128, 2 * W], dt.float16,
                                      tag=f"ftp{p}", name=f"ftp{p}")
                        nc.scalar.activation(ft[:], psp[:], AF.Tanh)
                        fts.append(ft)

                    # tails: pair0's chain on DVE (its state feeds the next
                    # step's early matmuls); pair1's blend on GPSIMD
                    for p in range(2):
                        ft = fts[p]
                        if NREC == 2:
                            u = p2c.tile([128, 32], dt.float16,
                                         tag=f"up{p}", name=f"up{p}")
                            if mix_lo:
                                nc.vector.scalar_tensor_tensor(
                                    u[:], ft[:, 32:64], mixc, ft[:, 0:32],
                                    ALU.mult, ALU.add)
                            else:
                                nc.vector.scalar_tensor_tensor(
                                    u[:], ft[:, 0:32], mixc, ft[:, 32:64],
                                    ALU.mult, ALU.add)
                        else:
                            u = ft
                        eng = nc.gpsimd if p == 0 else nc.vector
                        m2h = m2a if p == 0 else m2b
                        dst = stgA if p == 0 else stgB
                        mc = p2c.tile([128, 32], dt.float16,
                                      tag=f"mcp{p}", name=f"mcp{p}")
                        eng.tensor_tensor(
                            mc[:], u[:, 0:32],
                            gat[:, tt * 128 + p * 32:tt * 128 + p * 32 + 32],
                            ALU.mult)
                        eng.tensor_tensor(
                            dst[:, tt * 32:(tt + 1) * 32],
                            mc[:], m2h[:], ALU.add)

                    # next step's (1-g)*s after the critical Pool ops
                    if tt < 15:
                        m2a, m2b = emit_m2(stgA, tt * 32, stgB, tt * 32,
                                           tt + 1)

                    # projection work in the PE stall window at step end
                    if tt % 2 == 1 and tt // 2 < len(units):
                        p1_unit(tile_idx, units[tt // 2])
                    if tt == 14 and g % 2 == 1:
                        p1_flush(tile_idx)
                        p1_prefetch(tile_idx + 1)

                stgd_r = stg_d[:].rearrange("p (t h c) -> p t h c",
                                            t=T, h=2)[:, g * 16:(g + 1) * 16]
                nc.sync.dma_start(
                    stgd_r[:, :, 0],
                    stgA[:].rearrange("p (t c) -> p t c", t=16))
                nc.sync.dma_start(
                    stgd_r[:, :, 1],
                    stgB[:].rearrange("p (t c) -> p t c", t=16))
                prev_A, prev_B = stgA, stgB

    nc.compile()
    return nc


def _pack_lhsT_blocks(W, kdim, mdim, dtype):
    """W: [mdim*128, kdim*128]; returns [128, kdim*mdim*128] with block
    (k, j) at cols (k*mdim+j)*128 equal to W[j-chunk, k-chunk].T."""
    nk, nj = kdim, mdim
    out = np.zeros((128, nk * nj * 128), dtype=dtype)
    for k in range(nk):
        for j in range(nj):
            blk = W[j * 128:(j + 1) * 128, k * 128:(k + 1) * 128].T
            out[:, (k * nj + j) * 128:(k * nj + j + 1) * 128] = blk
    return np.ascontiguousarray(out)


def kernel(x_seq, s0, A0_w, B0_w, B0_b, A1_w, B1_w, B1_b, gate_w, gate_b,
           alpha, z, _T=None, _trace=False):
    from concourse.bass_utils import run_bass_kernel_spmd

    T = int(_T or T_FULL)
    alpha_f = float(np.asarray(alpha))
    z_i = int(np.asarray(z))

    key = (alpha_f, z_i, T)
    if key not in _cache:
        _cache[key] = _build(alpha_f, z_i, T)
    nc = _cache[key]

    NREC = 2 if z_i != 0 else 1
    NMAT = NREC + 1

    x_seq = np.asarray(x_seq, dtype=np.float32)
    s0 = np.asarray(s0, dtype=np.float32)

    # ---- shared (replicated) weight packing ----
    mats = [np.asarray(B0_w), np.asarray(B1_w), np.asarray(gate_w)] \
        if z_i != 0 else [np.asarray(B0_w), np.asarray(gate_w)]
    biases = [np.asarray(B0_b), np.asarray(B1_b), np.asarray(gate_b)] \
        if z_i != 0 else [np.asarray(B0_b), np.asarray(gate_b)]
    pw = np.concatenate(
        [_pack_lhsT_blocks(W.astype(np.float32), 2, 4, np.float32)
         for W in mats], axis=1)
    pw = np.ascontiguousarray(pw)

    bias = np.zeros((128, 4 * NMAT + 4), np.float32)
    for mi, bvec in enumerate(biases):
        bias[:, mi * 4:(mi + 1) * 4] = bvec.astype(np.float32).reshape(4, 128).T
    # negated gate bias (for g1m = sigmoid(-x) on ACT with scale=-1)
    bias[:, 4 * NMAT:] = -bias[:, (NMAT - 1) * 4:NMAT * 4]

    recs = [np.asarray(A0_w)] if z_i == 0 else [np.asarray(A0_w), np.asarray(A1_w)]
    aw = np.concatenate(
        [_pack_lhsT_blocks(A.astype(np.float32), 4, 4, np.float32)
         for A in recs], axis=1).astype(np.float16)
    aw = np.ascontiguousarray(aw)

    IDEN = np.ascontiguousarray(np.eye(128, dtype=np.float16))

    # ---- per-core inputs ----
    in_maps = []
    for c in range(N_CORES):
        bc = c * B_LOC
        xc = x_seq[bc:bc + B_LOC, :T]                       # [16, T, 256]
        xT = np.ascontiguousarray(
            xc.transpose(2, 1, 0).reshape(2, 128, T * B_LOC))
        s0c = s0[bc:bc + B_LOC]                             # [16, 512]
        s0T = np.ascontiguousarray(
            s0c.T.reshape(4, 128, B_LOC).transpose(1, 0, 2).reshape(128, 64)
        ).astype(np.float16)
        in_maps.append({
            "xT": xT, "pw": pw, "bias": bias, "aw": aw, "s0T": s0T,
            "iden": IDEN,
        })

    res = run_bass_kernel_spmd(nc, in_maps, list(range(N_CORES)), trace=_trace)
    if _trace:
        kernel._last_res = res

    out = np.empty((B_FULL, T + 1, S_DIM), np.float32)
    for c in range(N_CORES):
        bc = c * B_LOC
        stg = res.results[c]["stg"]                         # [128, T*64] fp16
        out[bc:bc + B_LOC, 0] = s0[bc:bc + B_LOC]
        out[bc:bc + B_LOC, 1:] = (
            stg.reshape(128, T, 4, B_LOC).transpose(3, 1, 2, 0)
            .reshape(B_LOC, T, S_DIM).astype(np.float32))
    return out


# revision 36
# speedup vs baseline: 1.0624x; 1.0624x over previous
"""Trainium2 Bass kernel for the BinaryMechanismSSM problem.

Full inputs in, full outputs out. Internally: batch (128) sharded 8 ways
(16 rows/core). Per core, a single fused pass:
  Projections (phase 1) are interleaved into the recurrence's idle PE/DVE/
  ACT windows: per 512-token tile, f32r matmuls compute bx0/bx1 (bias add
  + fp16 cast on DVE) and the gate planes gco = gcoef*sigmoid(x) (Pool)
  and g1m = sigmoid(-x) (ACT), packed into per-group-layout SBUF tiles and
  shipped to DRAM staging with one DMA each.
  Recurrence: T sequential steps, feature-block (j) pipelined. State lives
  as fp16 slices of a per-group staging tile stg_sb[p, t*64+j*16+b]
  (s[b, 128j+p] after step t). Per step: 4 psum tiles [128, 2*16] (one per
  feature block j); 1 fp16 identity matmul injects bx, 8 fp16 A-matmuls
  accumulate (each block consumes state block k in cyclic order ending
  with k=j); per-block tail tanh (ACT) -> mix/blend. Late-closing blocks
  j2/j3 run their blend on DVE, early blocks j0/j1 on GPSIMD; the (1-g)*s
  term is computed per half on the engine that produced that state half.
  One DMA per group ships 16 steps of states to DRAM; host re-layouts to
  [B, T+1, S].
"""
import numpy as np

B_FULL = 128
T_FULL = 1024
I_DIM = 256
S_DIM = 512
N_CORES = 8
B_LOC = B_FULL // N_CORES  # 16

_cache = {}


def _build(alpha: float, z: int, T: int):
    import concourse.bass as bass
    from concourse import bacc
    import concourse.mybir as mybir
    from concourse.tile import TileContext

    dt = mybir.dt
    AF = mybir.ActivationFunctionType
    ALU = mybir.AluOpType

    TOK = T * B_LOC          # tokens per core
    NTT = TOK // 512         # phase-1 token tiles (32 steps each)
    NG = T // 16             # phase-2 step groups
    NREC = 2 if z != 0 else 1
    NMAT = NREC + 1
    W = NREC * 16            # psum tile width per j block
    LAG = 2                  # tiles of projection lead

    # gate fold: st = gco * u + g1m * s, gco = gcoef * sigmoid, with
    #   alpha <= 0.5: gcoef = 1-alpha, u = ft0 + gam*ft1, gam = a/(1-a)
    #   alpha >  0.5: gcoef = alpha,   u = bet*ft0 + ft1, bet = (1-a)/a
    if NREC == 1:
        gcoef, mixc, mix_lo = 1.0, None, None
    elif alpha <= 0.5:
        gcoef, mixc, mix_lo = 1.0 - alpha, alpha / (1.0 - alpha), True
    else:
        gcoef, mixc, mix_lo = alpha, (1.0 - alpha) / alpha, False

    nc = bacc.Bacc("TRN2", target_bir_lowering=False, debug=False,
                   num_devices=N_CORES)

    xT_d = nc.declare_dram_parameter("xT", [2, 128, TOK], dt.float32r, isOutput=False)
    pw_d = nc.declare_dram_parameter("pw", [128, NMAT * 2 * 4 * 128], dt.float32r, isOutput=False)
    bias_d = nc.declare_dram_parameter("bias", [128, 4 * NMAT + 4], dt.float32, isOutput=False)
    aw_d = nc.declare_dram_parameter("aw", [128, NREC * 16 * 128], dt.float16, isOutput=False)
    s0_d = nc.declare_dram_parameter("s0T", [128, 64], dt.float16, isOutput=False)
    iden_d = nc.declare_dram_parameter("iden", [128, 128], dt.float16, isOutput=False)
    stg_d = nc.declare_dram_parameter("stg", [128, T * 64], dt.float16, isOutput=True)

    with TileContext(nc) as tc:
      with tc.tile_pool(name="dram", bufs=1, space="DRAM") as dpool:
        # bx staging: col = t*(NREC*64) + m*64 + j*16 + b
        pjb_d = dpool.tile([128, T * NREC * 64], dt.float16, tag="pjbd",
                           name="pjbd")
        # gate staging: col = t*128 + kind*64 + j*16 + b  (0=gco, 1=g1m)
        gat_d = dpool.tile([128, T * 128], dt.float16, tag="gatd", name="gatd")

        with (
            tc.tile_pool(name="wp", bufs=1) as wp,
            tc.tile_pool(name="p1x", bufs=3) as p1x,
            tc.tile_pool(name="p1o", bufs=3) as p1o,
            tc.tile_pool(name="p1ps", bufs=3, space="PSUM") as p1ps,
            tc.tile_pool(name="p2in", bufs=2) as p2in,
            tc.tile_pool(name="p2stg", bufs=2) as p2stg,
            tc.tile_pool(name="p2c", bufs=16) as p2c,
            tc.tile_pool(name="p2ps", bufs=2, space="PSUM") as p2ps,
        ):
            pw = wp.tile([128, NMAT * 2 * 4 * 128], dt.float32r)
            nc.sync.dma_start(pw[:], pw_d[:])
            bias = wp.tile([128, 4 * NMAT + 4], dt.float32)
            nc.sync.dma_start(bias[:], bias_d[:])
            gcoef_t = wp.tile([128, 256], dt.float16)
            nc.gpsimd.memset(gcoef_t[:], gcoef)
            aw = wp.tile([128, NREC * 16 * 128], dt.float16)
            nc.sync.dma_start(aw[:], aw_d[:])
            iden = wp.tile([128, 128], dt.float16)
            nc.sync.dma_start(iden[:], iden_d[:])
            s0sb = wp.tile([128, 64], dt.float16)
            nc.sync.dma_start(s0sb[:], s0_d[:])

            # ---------------- projection emitters ----------------
            p1st = {}

            def p1_prefetch(tt):
                if tt >= NTT:
                    return
                xt = p1x.tile([128, 2 * 512], dt.float32r, tag="xt")
                for i in range(2):
                    nc.sync.dma_start(xt[:, i * 512:(i + 1) * 512],
                                      xT_d[i, :, tt * 512:(tt + 1) * 512])
                p1st[tt] = {"xt": xt}

            def p1_unit(tt, u):
                """u in 0..NMAT*4-1 -> (mat, j): 2 matmuls + post ops."""
                if tt >= NTT:
                    return
                st = p1st[tt]
                if u == 0:
                    st["pjbpk"] = p1o.tile([128, 32 * NREC * 64], dt.float16,
                                           tag="pjbpk", name="pjbpk")
                    st["gatpk"] = p1o.tile([128, 32 * 128], dt.float16,
                                           tag="gatpk", name="gatpk")
                mat, j = divmod(u, 4)
                pjbpk_r = st["pjbpk"][:].rearrange(
                    "p (t m j b) -> p t m j b", t=32, m=NREC, j=4)
                gatpk_r = st["gatpk"][:].rearrange(
                    "p (t k j b) -> p t k j b", t=32, k=2, j=4)
                ps = p1ps.tile([128, 512], dt.float32, tag="pps")
                for i in range(2):
                    blk = ((mat * 2 + i) * 4 + j) * 128
                    nc.tensor.matmul(ps[:], pw[:, blk:blk + 128],
                                     st["xt"][:, i * 512:(i + 1) * 512],
                                     start=(i == 0), stop=(i == 1))
                psr = ps[:].rearrange("p (t b) -> p t b", t=32)
                bj = bias[:, mat * 4 + j:mat * 4 + j + 1]
                if mat == NMAT - 1:
                    g16 = p1o.tile([128, 512], dt.float16, tag="g16")
                    g16r = g16[:].rearrange("p (t b) -> p t b", t=32)
                    nbj = bias[:, 4 * NMAT + j:4 * NMAT + j + 1]
                    for h in range(2):
                        hs = slice(h * 16, (h + 1) * 16)
                        nc.scalar.activation(g16r[:, hs], psr[:, hs],
                                             AF.Sigmoid, bias=bj, scale=1.0)
                        nc.scalar.activation(gatpk_r[:, hs, 1, j, :],
                                             psr[:, hs], AF.Sigmoid,
                                             bias=nbj, scale=-1.0)
                        nc.gpsimd.tensor_tensor(
                            gatpk_r[:, hs, 0, j, :], g16r[:, hs],
                            gcoef_t[:].rearrange("p (t b) -> p t b", t=16),
                            ALU.mult)
                else:
                    for h in range(2):
                        hs = slice(h * 16, (h + 1) * 16)
                        nc.vector.tensor_scalar(pjbpk_r[:, hs, mat, j, :],
                                                psr[:, hs], bj, None, ALU.add)

            def p1_flush(tt):
                if tt >= NTT:
                    return
                st = p1st.pop(tt)
                GW32 = 32 * NREC * 64
                nc.sync.dma_start(pjb_d[:, tt * GW32:(tt + 1) * GW32],
                                  st["pjbpk"][:])
                nc.sync.dma_start(gat_d[:, tt * 4096:(tt + 1) * 4096],
                                  st["gatpk"][:])

            # prologue: first LAG tiles of projections
            NU = NMAT * 4
            for tt in range(min(LAG, NTT)):
                p1_prefetch(tt)
                for u in range(NU):
                    p1_unit(tt, u)
                p1_flush(tt)
            p1_prefetch(LAG)

            # ---------------- recurrence ----------------
            GW = 16 * NREC * 64
            prev_A = prev_B = None
            for g in range(NG):
                pjb = p2in.tile([128, GW], dt.float16, tag="pjb")
                nc.sync.dma_start(pjb[:], pjb_d[:, g * GW:(g + 1) * GW])
                pjb_r = pjb[:].rearrange("p (t m j b) -> p m j t b",
                                         t=16, m=NREC, j=4)
                gat = p2in.tile([128, 2048], dt.float16, tag="gat")
                nc.sync.dma_start(gat[:], gat_d[:, g * 2048:(g + 1) * 2048])

                # state halves in separate tiles so a read of one half never
                # waits on the other half's write
                stgA = p2stg.tile([128, 16 * 32], dt.float16, tag="stgA")
                stgB = p2stg.tile([128, 16 * 32], dt.float16, tag="stgB")

                def emit_m2(srcA, offA, srcB, offB, t):
                    m2a = p2c.tile([128, 32], dt.float16, tag="m2a",
                                   name="m2a")
                    nc.gpsimd.tensor_tensor(
                        m2a[:], srcA[:, offA:offA + 32],
                        gat[:, t * 128 + 64:t * 128 + 96], ALU.mult)
                    m2b = p2c.tile([128, 32], dt.float16, tag="m2b",
                                   name="m2b")
                    nc.gpsimd.tensor_tensor(
                        m2b[:], srcB[:, offB:offB + 32],
                        gat[:, t * 128 + 96:t * 128 + 128], ALU.mult)
                    return m2a, m2b

                # projection work interleaved into this group
                tile_idx = g // 2 + LAG
                ubase = 0 if g % 2 == 0 else NU - NU // 2
                units = list(range(ubase, min(ubase + NU - NU // 2, NU)))

                if g == 0:
                    srcA, offA, srcB, offB = s0sb, 0, s0sb, 32
                else:
                    srcA, offA, srcB, offB = prev_A, 15 * 32, prev_B, 15 * 32
                m2a, m2b = emit_m2(srcA, offA, srcB, offB, 0)

                for tt in range(16):
                    if tt > 0:
                        srcA, offA = stgA, (tt - 1) * 32
                        srcB, offB = stgB, (tt - 1) * 32

                    def st_rhs(k):
                        if k < 2:
                            return srcA[:, offA + k * 16:offA + (k + 1) * 16]
                        return srcB[:, offB + (k - 2) * 16:
                                    offB + (k - 1) * 16]

                    # two psum pairs, cols (m, jj, b); idens + early matmuls
                    # (k=0,1) first, then the late batch (k=2,3)
                    psps = []
                    for p in range(2):
                        psp = p2ps.tile([128, 2 * W], dt.float32,
                                        tag=f"psp{p}", name=f"psp{p}")
                        psps.append(psp)
                        nc.tensor.matmul(
                            psp[:].rearrange("q (m j b) -> q m j b",
                                             m=NREC, j=2),
                            iden[:], pjb_r[:, :, 2 * p:2 * p + 2, tt, :],
                            start=True, stop=False)
                        for jj in range(2):
                            j = 2 * p + jj
                            for m in range(NREC):
                                for k in (0, 1):
                                    blk = (m * 16 + k * 4 + j) * 128
                                    nc.tensor.matmul(
                                        psp[:, (m * 2 + jj) * 16:
                                            (m * 2 + jj + 1) * 16],
                                        aw[:, blk:blk + 128], st_rhs(k),
                                        start=False, stop=False)
                    fts = []
                    for p in range(2):
                        psp = psps[p]
                        for jj in range(2):
                            j = 2 * p + jj
                            for m in range(NREC):
                                for k in (2, 3):
                                    blk = (m * 16 + k * 4 + j) * 128
                                    nc.tensor.matmul(
                                        psp[:, (m * 2 + jj) * 16:
                                            (m * 2 + jj + 1) * 16],
                                        aw[:, blk:blk + 128], st_rhs(k),
                                        start=False, stop=(k == 3))
                        ft = p2c.tile([128, 2 * W], dt.float16,
                                      tag=f"ftp{p}", name=f"ftp{p}")
                        nc.scalar.activation(ft[:], psp[:], AF.Tanh)
                        fts.append(ft)

                    # tails: pair0's chain on DVE (its state feeds the next
                    # step's early matmuls); pair1's blend on GPSIMD
                    for p in range(2):
                        ft = fts[p]
                        if NREC == 2:
                            u = p2c.tile([128, 32], dt.float16,
                                         tag=f"up{p}", name=f"up{p}")
                            if mix_lo:
                                nc.vector.scalar_tensor_tensor(
                                    u[:], ft[:, 32:64], mixc, ft[:, 0:32],
                                    ALU.mult, ALU.add)
                            else:
                                nc.vector.scalar_tensor_tensor(
                                    u[:], ft[:, 0:32], mixc, ft[:, 32:64],
                                    ALU.mult, ALU.add)
                        else:
                            u = ft
                        eng = nc.vector if p == 0 else nc.gpsimd
                        m2h = m2a if p == 0 else m2b
                        dst = stgA if p == 0 else stgB
                        mc = p2c.tile([128, 32], dt.float16,
                                      tag=f"mcp{p}", name=f"mcp{p}")
                        eng.tensor_tensor(
                            mc[:], u[:, 0:32],
                            gat[:, tt * 128 + p * 32:tt * 128 + p * 32 + 32],
                            ALU.mult)
                        eng.tensor_tensor(
                            dst[:, tt * 32:(tt + 1) * 32],
                            mc[:], m2h[:], ALU.add)

                    # next step's (1-g)*s after the critical Pool ops
                    if tt < 15:
                        m2a, m2b = emit_m2(stgA, tt * 32, stgB, tt * 32,
                                           tt + 1)

                    # projection work in the PE stall window at step end
                    if tt % 2 == 1 and tt // 2 < len(units):
                        p1_unit(tile_idx, units[tt // 2])
                    if tt == 14 and g % 2 == 1:
                        p1_flush(tile_idx)
                        p1_prefetch(tile_idx + 1)

                stgd_r = stg_d[:].rearrange("p (t h c) -> p t h c",
                                            t=T, h=2)[:, g * 16:(g + 1) * 16]
                nc.sync.dma_start(
                    stgd_r[:, :, 0],
                    stgA[:].rearrange("p (t c) -> p t c", t=16))
                nc.sync.dma_start(
                    stgd_r[:, :, 1],
                    stgB[:].rearrange("p (t c) -> p t c", t=16))
                prev_A, prev_B = stgA, stgB

    nc.compile()
    return nc


def _pack_lhsT_blocks(W, kdim, mdim, dtype):
    """W: [mdim*128, kdim*128]; returns [128, kdim*mdim*128] with block
    (k, j) at cols (k*mdim+j)*128 equal to W[j-chunk, k-chunk].T."""
    nk, nj = kdim, mdim
    out = np.zeros((128, nk * nj * 128), dtype=dtype)
    for k in range(nk):
        for j in range(nj):
            blk = W[j * 128:(j + 1) * 128, k * 128:(k + 1) * 128].T
            out[:, (k * nj + j) * 128:(k * nj + j + 1) * 128] = blk
    return np.ascontiguousarray(out)


def kernel(x_seq, s0, A0_w, B0_w, B0_b, A1_w, B1_w, B1_b, gate_w, gate_b,
           alpha, z, _T=None, _trace=False):
    from concourse.bass_utils import run_bass_kernel_spmd

    T = int(_T or T_FULL)
    alpha_f = float(np.asarray(alpha))
    z_i = int(np.asarray(z))

    key = (alpha_f, z_i, T)
    if key not in _cache:
        _cache[key] = _build(alpha_f, z_i, T)
    nc = _cache[key]

    NREC = 2 if z_i != 0 else 1
    NMAT = NREC + 1

    x_seq = np.asarray(x_seq, dtype=np.float32)
    s0 = np.asarray(s0, dtype=np.float32)

    # ---- shared (replicated) weight packing ----
    mats = [np.asarray(B0_w), np.asarray(B1_w), np.asarray(gate_w)] \
        if z_i != 0 else [np.asarray(B0_w), np.asarray(gate_w)]
    biases = [np.asarray(B0_b), np.asarray(B1_b), np.asarray(gate_b)] \
        if z_i != 0 else [np.asarray(B0_b), np.asarray(gate_b)]
    pw = np.concatenate(
        [_pack_lhsT_blocks(W.astype(np.float32), 2, 4, np.float32)
         for W in mats], axis=1)
    pw = np.ascontiguousarray(pw)

    bias = np.zeros((128, 4 * NMAT + 4), np.float32)
    for mi, bvec in enumerate(biases):
        bias[:, mi * 4:(mi + 1) * 4] = bvec.astype(np.float32).reshape(4, 128).T
    # negated gate bias (for g1m = sigmoid(-x) on ACT with scale=-1)
    bias[:, 4 * NMAT:] = -bias[:, (NMAT - 1) * 4:NMAT * 4]

    recs = [np.asarray(A0_w)] if z_i == 0 else [np.asarray(A0_w), np.asarray(A1_w)]
    aw = np.concatenate(
        [_pack_lhsT_blocks(A.astype(np.float32), 4, 4, np.float32)
         for A in recs], axis=1).astype(np.float16)
    aw = np.ascontiguousarray(aw)

    IDEN = np.ascontiguousarray(np.eye(128, dtype=np.float16))

    # ---- per-core inputs ----
    in_maps = []
    for c in range(N_CORES):
        bc = c * B_LOC
        xc = x_seq[bc:bc + B_LOC, :T]                       # [16, T, 256]
        xT = np.ascontiguousarray(
            xc.transpose(2, 1, 0).reshape(2, 128, T * B_LOC))
        s0c = s0[bc:bc + B_LOC]                             # [16, 512]
        s0T = np.ascontiguousarray(
            s0c.T.reshape(4, 128, B_LOC).transpose(1, 0, 2).reshape(128, 64)
        ).astype(np.float16)
        in_maps.append({
            "xT": xT, "pw": pw, "bias": bias, "aw": aw, "s0T": s0T,
            "iden": IDEN,
        })

    res = run_bass_kernel_spmd(nc, in_maps, list(range(N_CORES)), trace=_trace)
    if _trace:
        kernel._last_res = res

    out = np.empty((B_FULL, T + 1, S_DIM), np.float32)
    for c in range(N_CORES):
        bc = c * B_LOC
        stg = res.results[c]["stg"]                         # [128, T*64] fp16
        out[bc:bc + B_LOC, 0] = s0[bc:bc + B_LOC]
        out[bc:bc + B_LOC, 1:] = (
            stg.reshape(128, T, 4, B_LOC).transpose(3, 1, 2, 0)
            .reshape(B_LOC, T, S_DIM).astype(np.float32))
    return out
